# revision 11
# baseline (speedup 1.0000x reference)
"""Trainium2 Bass kernel for nn_ComplexMultiheadAttention.

Problem (reference.py): complex multihead attention,
  B=2, N=1024, D=1024, HEADS=16, dim_head=64.
  q/k/v = complex linear projections of x = x_real + i*x_imag,
  4 softmax-attention combos g0..g3 over (q-part, k-part, v-part),
  sign-combined into o_real/o_imag, then a complex output projection.
  Output: [2, B, N, D] fp32 (real, imag).

Sharding (8 NeuronCores): core c = (b = c // 4) x (head group hg = c % 4,
4 heads each). Each core computes projections + attention + sign-combine
for its 4 heads and a partial output projection (its heads' contribution,
full output columns); the host unshards by summing the 4 partials per
batch.

Schedule (single fused stream, ScalarE exp is the pacing engine):
- bf16 host-prepped pre-tiled inputs; DMAs emitted in consumption order.
- pair-0 q/k projections (8 concurrent K-chains) -> v -> attention starts.
- pair-1 q/k projections are interleaved INTO the attention stream (one
  half-chain per attention unit) so the PE's spare capacity under the
  exp-paced stream absorbs them with no serial phase.
- QK^T runs as two CONCURRENT K=64 matmuls in distinct row groups
  (tile_position (0,0)/(64,0) via partition offsets) - half the PE time
  of the K=128 duplicated form; the duplicated q/k layout is kept so
  both row groups have their operands.
- softmax without max-subtraction; exp on ScalarE straight out of PSUM;
  denominator via a ones-row appended to V; one [8,512] reciprocal per
  head; the per-head normalize is spread ONE (broadcast, multiply) piece
  per subsequent attention unit so no engine-queue lump ever blocks
  av-PSUM recycling.
- output projection weights prefetched in phase A; phase C chains emit
  heads 0-2 first (prestart during the last head's normalize tail) and
  head 3 last.
"""

import ml_dtypes
import numpy as np

import concourse.mybir as mybir
import concourse.tile as tile
from concourse import bacc
from concourse.bass_utils import run_bass_kernel_spmd

P = 128
NTOK = 1024  # tokens per batch
KD = 16  # k-tiles over the stacked 2048 contraction dim
CD = 64  # dim per head
HL = 4  # heads per core
F32 = mybir.dt.float32
BF16 = mybir.dt.bfloat16
EXP = mybir.ActivationFunctionType.Exp
SCALE = float(CD) ** -0.5
BF = ml_dtypes.bfloat16

_nc_cache = None

WNAMES = ["wqr", "wqi", "wkr", "wki"]


def _build():
    nc = bacc.Bacc("TRN2", target_bir_lowering=False, debug=False, num_devices=8)

    x = nc.declare_dram_parameter("x", [P, KD, NTOK], BF16, isOutput=False)
    wd = {
        n: nc.declare_dram_parameter(n, [P, 2, KD, P], BF16, isOutput=False)
        for n in WNAMES
    }
    wv = nc.declare_dram_parameter("wv", [P, KD, 512], BF16, isOutput=False)
    wyr = nc.declare_dram_parameter("wyr", [P, HL, NTOK], BF16, isOutput=False)
    wyi = nc.declare_dram_parameter("wyi", [P, HL, NTOK], BF16, isOutput=False)
    yp = nc.declare_dram_parameter("ypart", [2, NTOK, 1024], F32, isOutput=True)

    with tile.TileContext(nc) as tc:
        with (
            tc.tile_pool(name="persist", bufs=1) as pp,
            tc.tile_pool(name="small", bufs=2) as sp,
        ):
            # q^T/k^T duplicated along partitions: [128 = head d(64) twice,
            # head, tokens]; QK uses the two halves as two concurrent K=64
            # row-group matmuls.
            qrT = pp.tile([P, HL, NTOK], BF16, tag="qrT")
            qiT = pp.tile([P, HL, NTOK], BF16, tag="qiT")
            krT = pp.tile([P, HL, NTOK], BF16, tag="krT")
            kiT = pp.tile([P, HL, NTOK], BF16, tag="kiT")
            # V with ones column appended: [tok-tile, jt, head, 65]
            vhat_r = pp.tile([P, 8, HL, CD + 1], BF16, tag="vhr")
            vhat_i = pp.tile([P, 8, HL, CD + 1], BF16, tag="vhi")
            # combined attention output, per-head tiles so phase C's
            # head-0..2 matmuls don't false-depend on head 3's write
            Ots = [
                pp.tile([P, NTOK], BF16, tag=f"O{h}", name=f"O{h}")
                for h in range(HL)
            ]
            # phase-C weights, prefetched during phase A
            wyr_sb = pp.tile([P, HL, NTOK], BF16, tag="wyr")
            wyi_sb = pp.tile([P, HL, NTOK], BF16, tag="wyi")

            pak_cm = tc.tile_pool(name="pa_keep", bufs=1)
            pak = pak_cm.__enter__()
            if True:
                xs = pak.tile([P, KD, NTOK], BF16, tag="xs")
                wts1 = {
                    n: pak.tile([P, KD, P], BF16, tag=f"{n}1", name=f"wt_{n}1")
                    for n in WNAMES
                }
                with tc.tile_pool(name="pa_tmp", bufs=1) as pat:
                    wts0 = {
                        n: pat.tile([P, KD, P], BF16, tag=f"{n}0", name=f"wt_{n}0")
                        for n in WNAMES
                    }
                    wvt = pat.tile([P, KD, 512], BF16, tag="wv")
                    ones = pat.tile([P, 8, HL, 1], F32, tag="ones")

                    # DMA in consumption order
                    nc.sync.dma_start(wts0["wqr"][:], wd["wqr"][:, 0])
                    for i, n in enumerate(["wqi", "wkr", "wki"]):
                        nc.sync.dma_start(
                            xs[:, 4 * i : 4 * i + 4, :], x[:, 4 * i : 4 * i + 4, :]
                        )
                        nc.sync.dma_start(wts0[n][:], wd[n][:, 0])
                    nc.sync.dma_start(xs[:, 12:16, :], x[:, 12:16, :])
                    nc.sync.dma_start(wvt[:], wv[:])
                    for n in WNAMES:
                        nc.sync.dma_start(wts1[n][:], wd[n][:, 1])
                    nc.sync.dma_start(wyr_sb[:], wyr[:])
                    nc.sync.dma_start(wyi_sb[:], wyi[:])
                    nc.vector.memset(ones[:], 1.0)
                    nc.vector.tensor_copy(vhat_r[:, :, :, CD : CD + 1], ones[:])
                    nc.vector.tensor_copy(vhat_i[:, :, :, CD : CD + 1], ones[:])

                    # ---- pair-0 q/k projections: 8 concurrent K-chains ----
                    with tc.tile_pool(name="pa_ps", bufs=8, space="PSUM") as paps:
                        pss = {
                            (n, tch): paps.tile(
                                [P, 512], F32, tag="proj", name="ps"
                            )
                            for n in WNAMES
                            for tch in range(2)
                        }
                        for kt in range(KD):
                            for n in WNAMES:
                                for tch in range(2):
                                    nc.tensor.matmul(
                                        pss[(n, tch)][:],
                                        wts0[n][:, kt, :],
                                        xs[:, kt, tch * 512 : (tch + 1) * 512],
                                        start=(kt == 0),
                                        stop=(kt == KD - 1),
                                    )
                        for n, dstT in zip(WNAMES, (qrT, qiT, krT, kiT)):
                            for tch in range(2):
                                ps = pss[(n, tch)]
                                sl = slice(tch * 512, (tch + 1) * 512)
                                nc.vector.tensor_copy(dstT[0:CD, 0, sl], ps[0:CD, :])
                                nc.vector.tensor_copy(dstT[CD:P, 1, sl], ps[CD:P, :])
                            nc.gpsimd.dma_start(dstT[CD:P, 0, :], dstT[0:CD, 0, :])
                            nc.gpsimd.dma_start(dstT[0:CD, 1, :], dstT[CD:P, 1, :])

                        # ---- v projections in the same pool: each v
                        # chain ring-aliases one evacuated pair-0 slot ----
                        for half in range(2):
                            psv = [
                                paps.tile([P, 512], F32, tag="proj", name="psv")
                                for _ in range(4)
                            ]
                            for kt in range(KD):
                                for j in range(4):
                                    tt = half * 4 + j
                                    nc.tensor.matmul(
                                        psv[j][:],
                                        xs[:, kt, tt * 128 : (tt + 1) * 128],
                                        wvt[:, kt, :],
                                        start=(kt == 0),
                                        stop=(kt == KD - 1),
                                    )
                            for j in range(4):
                                tt = half * 4 + j
                                nc.vector.tensor_copy(
                                    vhat_r[:, tt, :, 0:CD],
                                    psv[j][:, 0:256].rearrange(
                                        "p (h d) -> p h d", d=CD
                                    ),
                                )
                                nc.vector.tensor_copy(
                                    vhat_i[:, tt, :, 0:CD],
                                    psv[j][:, 256:512].rearrange(
                                        "p (h d) -> p h d", d=CD
                                    ),
                                )
                # pa_tmp closed

                # ---------------- attention + interleaved pair-1 ----------------
                _cms = [
                    tc.tile_pool(name="pb_on", bufs=1),
                    tc.tile_pool(name="pb_c", bufs=1),
                    tc.tile_pool(name="pb_oav", bufs=8),
                    tc.tile_pool(name="pb_nrm", bufs=3),
                    tc.tile_pool(name="pb_den", bufs=2),
                ]
                onpool, cpool, oavp, nrm, dpool = [c.__enter__() for c in _cms]
                with (
                    tc.tile_pool(name="pb_pt", bufs=2) as ptpool,
                    tc.tile_pool(name="pb_s", bufs=3, space="PSUM") as spool,
                    tc.tile_pool(name="pb_ax", bufs=1, space="PSUM") as axp,
                ):
                    ons = {}

                    def get_on(h):
                        if h not in ons:
                            ons[h] = onpool.tile(
                                [CD, 4, NTOK], F32, tag=f"on{h % 2}", name="on"
                            )
                        return ons[h]

                    def emit_qk_exp(h, g, ic):
                        qT = qrT if g in (0, 1) else qiT
                        kT = krT if g in (0, 2) else kiT
                        pt = ptpool.tile([P, 8, 512], BF16, tag="pt", name="pt")
                        for u in range(4):
                            st = spool.tile([P, 1024], F32, tag="s", name="st")
                            jtA, jtB = 2 * u, 2 * u + 1
                            # two concurrent K=64 matmuls in row groups
                            # (0,0) and (64,0)
                            nc.tensor.matmul(
                                st[:, 0:512],
                                kT[0:CD, h, jtA * 128 : (jtA + 1) * 128],
                                qT[0:CD, h, ic * 512 : (ic + 1) * 512],
                                start=True,
                                stop=True,
                            )
                            nc.tensor.matmul(
                                st[:, 512:1024],
                                kT[CD:P, h, jtB * 128 : (jtB + 1) * 128],
                                qT[CD:P, h, ic * 512 : (ic + 1) * 512],
                                start=True,
                                stop=True,
                            )
                            nc.scalar.activation(
                                pt[:, 2 * u : 2 * u + 2, :].rearrange(
                                    "p a b -> p (a b)"
                                ),
                                st[:],
                                EXP,
                                scale=SCALE,
                            )
                        return pt

                    def emit_av(h, g, ic, pt, hstate):
                        vh = vhat_r if g in (0, 2) else vhat_i
                        av = axp.tile(
                            [CD + 1, 512], F32, tag="av", name="av", bufs=1
                        )
                        for jt in range(8):
                            nc.tensor.matmul(
                                av[:],
                                vh[:, jt, h, :],
                                pt[:, jt, :],
                                start=(jt == 0),
                                stop=(jt == 7),
                            )
                        oav = oavp.tile([CD + 1, 512], F32, tag="oav", name="oav")
                        nc.vector.tensor_copy(oav[:], av[:])
                        iu = g * 2 + ic
                        nc.gpsimd.dma_start(
                            hstate["den"][iu : iu + 1, :], oav[CD : CD + 1, :]
                        )
                        hstate["oavs"].append((g, ic, oav))

                    def emit_recip(h, hstate):
                        rp8 = sp.tile([8, 512], F32, tag="rp8")
                        nc.vector.reciprocal(rp8[:], hstate["den"][:])
                        hstate["rp8"] = rp8

                    def emit_piece(h, hstate, j):
                        g, ic, oav = hstate["oavs"][j]
                        iu = g * 2 + ic
                        rp1 = nrm.tile([1, 512], F32, tag="rp1")
                        nc.gpsimd.dma_start(rp1[:], hstate["rp8"][iu : iu + 1, :])
                        bc = nrm.tile([CD, 512], F32, tag="bc")
                        nc.gpsimd.partition_broadcast(bc[:], rp1[:])
                        nc.vector.tensor_mul(
                            get_on(h)[:, g, ic * 512 : (ic + 1) * 512],
                            oav[0:CD, :],
                            bc[:],
                        )

                    def emit_combine_a(h):
                        on = ons[h]
                        s = cpool.tile([CD, NTOK], F32, tag="cs", name="cs")
                        t = cpool.tile([CD, NTOK], F32, tag="ct", name="ct")
                        nc.vector.tensor_sub(s[:], on[:, 0, :], on[:, 3, :])
                        nc.vector.tensor_add(t[:], on[:, 1, :], on[:, 2, :])
                        return s, t

                    def emit_combine_b(h, s, t):
                        oi = cpool.tile([CD, NTOK], BF16, tag="oi", name="oi")
                        nc.vector.tensor_sub(Ots[h][0:CD, :], s[:], t[:])
                        nc.vector.tensor_add(oi[:], s[:], t[:])
                        nc.gpsimd.dma_start(Ots[h][CD:P, :], oi[:])

                    # pair-1 projection half-chains, one per early unit
                    p1_state = {"ps": None}

                    P1ORDER = ["wqr", "wkr", "wki", "wqi"]

                    def emit_p1_halfchain(hc):
                        c, second = hc // 2, hc % 2
                        n, tch = P1ORDER[c // 2], c % 2
                        if not second:
                            p1_state["ps"] = axp.tile(
                                [P, 512], F32, tag="proj", name="p1ps", bufs=1
                            )
                        ps = p1_state["ps"]
                        for kt in range(8 * second, 8 * second + 8):
                            nc.tensor.matmul(
                                ps[:],
                                wts1[n][:, kt, :],
                                xs[:, kt, tch * 512 : (tch + 1) * 512],
                                start=(kt == 0),
                                stop=(kt == KD - 1),
                            )
                        if second:
                            dstT = (qrT, qiT, krT, kiT)[WNAMES.index(n)]
                            sl = slice(tch * 512, (tch + 1) * 512)
                            nc.vector.tensor_copy(dstT[0:CD, 2, sl], ps[0:CD, :])
                            nc.vector.tensor_copy(dstT[CD:P, 3, sl], ps[CD:P, :])
                            if tch == 1:
                                nc.gpsimd.dma_start(
                                    dstT[CD:P, 2, :], dstT[0:CD, 2, :]
                                )
                                nc.gpsimd.dma_start(
                                    dstT[0:CD, 3, :], dstT[CD:P, 3, :]
                                )

                    units = [
                        (h, g, ic)
                        for h in range(HL)
                        for g in range(4)
                        for ic in range(2)
                    ]
                    hstates = {}

                    def ensure(h):
                        if h not in hstates:
                            hstates[h] = {
                                "den": dpool.tile(
                                    [8, 512], F32, tag="den", name="den"
                                ),
                                "oavs": [],
                            }
                        return hstates[h]

                    # per-iteration scheduled normalize work:
                    # iteration i -> list of thunks
                    sched = {}

                    def at(i, fn, *args):
                        sched.setdefault(i, []).append((fn, args))

                    combs = {}

                    def _comb_a(h):
                        combs[h] = emit_combine_a(h)

                    def _comb_b(h):
                        s, t = combs.pop(h)
                        emit_combine_b(h, s, t)

                    prev = None
                    n_it = len(units)
                    for i, unit in enumerate(units):
                        pt = emit_qk_exp(*unit)
                        if prev is not None:
                            ph = prev[0][0]
                            emit_av(*prev[0], prev[1], ensure(ph))
                            if prev[0][1:] == (3, 1):
                                # spread: recip, 8 pieces, combine over the
                                # next 10 iterations
                                hs = hstates.pop(ph)
                                at(i, emit_recip, ph, hs)
                                for j in range(8):
                                    at(i + 1 + j, emit_piece, ph, hs, j)
                                at(i + 9, _comb_a, ph)
                                at(i + 10, _comb_b, ph)
                        if 1 <= i <= 16:
                            emit_p1_halfchain(i - 1)
                        for fn, args in sched.pop(i, []):
                            fn(*args)
                        prev = (unit, pt)

                    # tail: AV of the last unit, then remaining scheduled work
                    ph = prev[0][0]
                    emit_av(*prev[0], prev[1], ensure(ph))
                    hs3 = hstates.pop(ph)
                    for i in sorted(sched):
                        for fn, args in sched.pop(i, []):
                            fn(*args)
                # attention pools closed

            # ---------------- Phase C: output projection ----------------
            # kt (=head) 0..2 first so chains prestart during the last
            # head's normalize tail; head 3 contribution last.
            with (
                tc.tile_pool(name="pc_ps", bufs=8, space="PSUM") as cps,
                tc.tile_pool(name="pc_o", bufs=8) as cop,
            ):
                # last head's normalize tail, emitted AFTER the C pools are
                # allocated so C's head-0..2 matmuls can prestart under it
                emit_recip(3, hs3)
                for j in range(8):
                    emit_piece(3, hs3, j)
                _comb_a(3)
                _comb_b(3)
                chains = [
                    (ri, tt, oc)
                    for ri in range(2)
                    for tt in range(8)
                    for oc in range(2)
                ]
                for b0 in range(0, 32, 8):
                    batch = chains[b0 : b0 + 8]
                    pss = []
                    for ri, tt, oc in batch:
                        ps = cps.tile([P, 512], F32, tag="y", name="psy")
                        pss.append(ps)
                        W = wyr_sb if ri == 0 else wyi_sb
                        for kt in range(3):
                            nc.tensor.matmul(
                                ps[:],
                                Ots[kt][:, tt * 128 : (tt + 1) * 128],
                                W[:, kt, oc * 512 : (oc + 1) * 512],
                                start=(kt == 0),
                                stop=False,
                            )
                    for (ri, tt, oc), ps in zip(batch, pss):
                        W = wyr_sb if ri == 0 else wyi_sb
                        nc.tensor.matmul(
                            ps[:],
                            Ots[3][:, tt * 128 : (tt + 1) * 128],
                            W[:, 3, oc * 512 : (oc + 1) * 512],
                            start=False,
                            stop=True,
                        )
                        ys = cop.tile([P, 512], F32, tag="ys")
                        nc.vector.tensor_copy(ys[:], ps[:])
                        nc.sync.dma_start(
                            yp[
                                ri,
                                tt * 128 : (tt + 1) * 128,
                                oc * 512 : (oc + 1) * 512,
                            ],
                            ys[:],
                        )
            for _cm in reversed(_cms):
                _cm.__exit__(None, None, None)
            pak_cm.__exit__(None, None, None)
    nc.compile()
    return nc


def _tile_k(a):
    # [K, M] -> [128, K//128, M]  (row k*128+p -> [p, k, m])
    K, M = a.shape
    return np.ascontiguousarray(a.reshape(K // P, P, M).transpose(1, 0, 2))


def _prep(inputs):
    f = np.float32
    xr = np.asarray(inputs["x_real"], f)
    xi = np.asarray(inputs["x_imag"], f)
    wq_r = np.asarray(inputs["wq_r"], f)
    wq_i = np.asarray(inputs["wq_i"], f)
    wkv_r = np.asarray(inputs["wkv_r"], f)
    wkv_i = np.asarray(inputs["wkv_i"], f)
    wout_r = np.asarray(inputs["wout_r"], f)
    wout_i = np.asarray(inputs["wout_i"], f)

    c = np.ascontiguousarray
    in_maps = []
    for core in range(8):
        b, hg = divmod(core, 4)
        c0 = hg * 256
        X = np.concatenate([xr[b].T, xi[b].T], axis=0)  # [2048, 1024]
        sl = slice(c0, c0 + 256)
        vsl = slice(1024 + c0, 1024 + c0 + 256)
        stacks = {
            "wqr": np.concatenate([wq_r[sl].T, -wq_i[sl].T], axis=0),
            "wqi": np.concatenate([wq_i[sl].T, wq_r[sl].T], axis=0),
            "wkr": np.concatenate([wkv_r[sl].T, -wkv_i[sl].T], axis=0),
            "wki": np.concatenate([wkv_i[sl].T, wkv_r[sl].T], axis=0),
        }
        m = {"x": _tile_k(X.astype(BF))}
        for n, W in stacks.items():
            # [2048, 256] -> [128, pair, kt, 128]
            t = W.reshape(KD, P, 2, P).transpose(1, 2, 0, 3)
            m[n] = c(t.astype(BF))
        WV = np.concatenate(
            [
                np.concatenate([wkv_r[vsl].T, -wkv_i[vsl].T], axis=0),
                np.concatenate([wkv_i[vsl].T, wkv_r[vsl].T], axis=0),
            ],
            axis=1,
        )
        m["wv"] = _tile_k(WV.astype(BF))
        Wyr = np.empty((512, 1024), f)
        Wyi = np.empty((512, 1024), f)
        for h in range(HL):
            cols = slice(c0 + h * CD, c0 + (h + 1) * CD)
            Wyr[h * P : h * P + CD] = wout_r[:, cols].T
            Wyr[h * P + CD : (h + 1) * P] = -wout_i[:, cols].T
            Wyi[h * P : h * P + CD] = wout_i[:, cols].T
            Wyi[h * P + CD : (h + 1) * P] = wout_r[:, cols].T
        m["wyr"] = _tile_k(Wyr.astype(BF))
        m["wyi"] = _tile_k(Wyi.astype(BF))
        in_maps.append(m)
    return in_maps


def _get_nc():
    global _nc_cache
    if _nc_cache is None:
        _nc_cache = _build()
    return _nc_cache


def _assemble(results):
    y = np.zeros((2, 2, NTOK, 1024), np.float32)
    for core in range(8):
        b = core // 4
        y[:, b] += results[core]["ypart"]
    return y


def run(inputs, trace=False, **kwargs):
    nc = _get_nc()
    in_maps = _prep(inputs)
    res = run_bass_kernel_spmd(
        nc, in_maps, core_ids=list(range(8)), trace=trace, **kwargs
    )
    return _assemble(res.results), res


def kernel(**inputs) -> np.ndarray:
    y, _ = run(inputs)
    return y
